# revision 1
# baseline (speedup 1.0000x reference)
"""Trainium2 Bass kernel for nn_BA_Model_46471546142978 (linear-attention fusion).

Self-contained: takes FULL inputs, shards batch across 8 NeuronCores (one
batch element per core), runs one SPMD Bass program, gathers FULL output.

Algorithm per core (batch b), restructured to avoid materializing K/V:
  A      = sum_n r_n * K[:,n] * low[:,n]^T          (8x64,  = Wk @ S)
  k_sum  = sum_n r_n * K[:,n] + eps                 (8,)
  matrix = A @ (gamma*Wv)^T                         (8x64)
  vs     = gamma*Wv @ u,  u = sum_n low[:,n]        (64,)
with r_n = 1/||Wk low_n||. The Q path runs at quarter resolution (128x128)
since nearest-upsample commutes with the per-pixel 1x1 convs + normalize:
  wv = gamma*(ms_pre + sq*vs) / (N*sq + d_pre),  sq = ||q_pre||
  out = low + upsample2x(wv)

Layout: "stacked halves" - partitions 0:63 = 64 channels of spatial top half
(rows Y<128), partitions 64:127 = bottom half. Full-res DVE/ACT ops run on
[128, *] tiles; matmuls run as concurrent tile_position pairs.
"""
from contextlib import ExitStack

import numpy as np
import ml_dtypes

import concourse.bass as bass
import concourse.bacc as bacc
import concourse.tile as tile
from concourse import mybir

FP = mybir.dt.float32
BF = mybir.dt.bfloat16
FR = mybir.dt.float32r
AF = mybir.ActivationFunctionType
ALU = mybir.AluOpType
AX = mybir.AxisListType

C = 64          # channels
CH = 128        # high channels
N = 65536       # full-res pixels (256x256)
NH = 32768      # per half
NQ = 16384      # quarter-res pixels (128x128)
NQH = 8192      # per half
W = 256         # full-res row width
WQ = 128        # quarter-res row width
CQ = 8          # q/k channels
EPS = 1e-6
BN_EPS = 1e-5

SEC = 8                 # sections for the low pass
SECN = NH // SEC        # 4096 half-pixels per section
SBLK = SECN // 128      # 32 transpose blocks per section
KP = 48                 # padded K partition count for DMA transpose (mult 16)
QCH = 512               # chunk size in quarter-res loops
PADW = WQ + 2           # 130, padded conv row
PADR = WQ // 2 + 2      # 66 padded rows per half
PADN = PADW * PADR      # per-half padded h1 buffer free size


def build_program(ctx: ExitStack, tc: tile.TileContext, outs, ins):
    nc = tc.nc
    (out_d,) = outs
    (low_d, high_d, whT_d, wup_d, bn_d, wqT_d, wkT_d, wvTg_d, iden_d) = ins

    consts = ctx.enter_context(tc.tile_pool(name="consts", bufs=1))
    resid = ctx.enter_context(tc.tile_pool(name="resid", bufs=1))

    # ---------------- constants ----------------
    whT = consts.tile([128, 128], BF)
    nc.sync.dma_start(whT[:], whT_d[:])
    wup = consts.tile([128, 9 * C], BF)
    nc.sync.dma_start(wup[:], wup_d[:])
    bnt = consts.tile([128, 2], FP)
    nc.sync.dma_start(bnt[:], bn_d[:])
    wqT = consts.tile([128, C], BF)
    nc.sync.dma_start(wqT[:], wqT_d[:])
    wkT = consts.tile([128, 32], BF)
    nc.sync.dma_start(wkT[:], wkT_d[:])
    wvTg = consts.tile([128, C], BF)
    nc.sync.dma_start(wvTg[:], wvTg_d[:])
    iden = consts.tile([128, 128], FP)
    nc.sync.dma_start(iden[:], iden_d[:])
    ones_bf = consts.tile([128, C], BF)
    nc.gpsimd.memset(ones_bf[:], 1.0)

    u_acc = consts.tile([128, SEC], FP)
    matrix2 = consts.tile([72, C], BF)    # rows 0:8 / 64:72 = gamma*matrix
    kore = consts.tile([72, C], BF)       # rows 0:8 / 64:72 = k_sum bcast cols
    vsrow = consts.tile([65, C], BF)      # rows 0 / 64 = gamma*vs/N

    lowf = resid.tile([128, NH], FP)      # resident low; becomes the output
    # stacked-halves DRAM views: [half, channel, n]
    low_hc = low_d.rearrange("c (h n) -> h c n", h=2)
    out_hc = out_d.rearrange("c (h n) -> h c n", h=2)

    # =========================================================
    # Phase 1: low pass - accumulate A [8,64], k_sum [8,1], u
    # =========================================================
    with tc.tile_pool(name="sp", bufs=2) as sp, \
         tc.tile_pool(name="sp1", bufs=1) as sp1, \
         tc.tile_pool(name="spp", bufs=2, space="PSUM") as spp, \
         tc.tile_pool(name="sacc", bufs=1, space="PSUM") as sacc:

        psA = sacc.tile([CQ, C], FP)
        psks = sacc.tile([1, 2 * SBLK * CQ], FP)
        ones_col = sp1.tile([128, 1], BF)
        nc.gpsimd.memset(ones_col[:], 1.0)

        n_mm = SEC * SBLK * 2
        mm_i = 0
        for s in range(SEC):
            lo = s * SECN
            hi = lo + SECN
            nc.sync.dma_start(lowf[:, lo:hi], low_hc[:, :, lo:hi])

            lowbf = sp.tile([128, SECN], BF, tag="lowbf")
            nc.scalar.activation(lowbf[:], lowf[:, lo:hi], AF.Copy,
                                 accum_out=u_acc[:, s:s + 1])

            kbf = sp.tile([KP, SECN], BF, tag="kbf")
            for cc in range(SECN // 1024):
                ps = spp.tile([KP, 1024], FP, tag="psK")
                for q2 in range(2):
                    sl5 = slice(q2 * QCH, (q2 + 1) * QCH)
                    sl = slice(cc * 1024 + q2 * QCH, cc * 1024 + (q2 + 1) * QCH)
                    nc.tensor.matmul(ps[0:32, sl5], wkT[0:64, 0:32],
                                     lowbf[0:64, sl], start=True, stop=True,
                                     skip_group_check=True,
                                     tile_position=(0, 0))
                    nc.tensor.matmul(ps[32:48, sl5], wkT[64:128, 0:16],
                                     lowbf[64:128, sl], start=True, stop=True,
                                     skip_group_check=True,
                                     tile_position=(64, 32))
                nc.vector.tensor_copy(kbf[0:KP, cc * 1024:(cc + 1) * 1024], ps[:])

            lowT = sp.tile([128, SBLK, 128], BF, tag="lowT")
            nc.sync.dma_start(lowT[:], lowbf[:], transpose=True)
            kT = sp.tile([128, SBLK, KP], BF, tag="kT")
            nc.sync.dma_start(kT[:], kbf[:], transpose=True)

            sqk = sp.tile([128, 2 * SBLK, CQ], BF, tag="sqk")
            nc.scalar.square(sqk[:, 0:SBLK, :], kT[:, :, 0:8])
            nc.scalar.square(sqk[:, SBLK:2 * SBLK, :], kT[:, :, 32:40])
            ksq = sp.tile([128, 2 * SBLK], FP, tag="ksq")
            nc.vector.tensor_reduce(ksq[:], sqk[:], axis=AX.X, op=ALU.add)
            sqr = sp.tile([128, 2 * SBLK], FP, tag="sqr")
            nc.scalar.sqrt(sqr[:], ksq[:])
            rr = sp.tile([128, 2 * SBLK], FP, tag="rr")
            nc.vector.reciprocal(rr[:], sqr[:])
            rrb = sp.tile([128, 2 * SBLK], BF, tag="rrb")
            nc.vector.tensor_copy(rrb[:], rr[:])

            ktr = sp.tile([128, 2 * SBLK, CQ], BF, tag="ktr")
            nc.vector.tensor_mul(
                ktr[:, 0:SBLK, :], kT[:, :, 0:8],
                rrb[:, 0:SBLK].unsqueeze(2).broadcast_to([128, SBLK, CQ]))
            nc.vector.tensor_mul(
                ktr[:, SBLK:2 * SBLK, :], kT[:, :, 32:40],
                rrb[:, SBLK:2 * SBLK].unsqueeze(2).broadcast_to([128, SBLK, CQ]))

            nc.tensor.matmul(psks[:], ones_col[:],
                             ktr[:].rearrange("p a b -> p (a b)"),
                             start=(s == 0), stop=(s == SEC - 1),
                             skip_group_check=True)
            for blk in range(SBLK):
                for half in range(2):
                    lhs = ktr[:, half * SBLK + blk, :]
                    rhs = lowT[:, blk, half * 64:(half + 1) * 64]
                    st = mm_i == 0
                    fin = mm_i == n_mm - 1
                    nc.tensor.matmul(psA[:], lhs, rhs, start=st, stop=fin,
                                     skip_group_check=True)
                    mm_i += 1

        # ----- small algebra -----
        asb = sp1.tile([CQ, C], FP)
        nc.vector.tensor_copy(asb[:], psA[:])
        psAT = sacc.tile([C, CQ], FP, tag="alg")
        nc.tensor.transpose(psAT[:], asb[:], iden[0:8, 0:8])
        atb = sp1.tile([C, CQ], BF)
        nc.scalar.copy(atb[:], psAT[:])

        psM = sacc.tile([72, C], FP, tag="alg")
        nc.tensor.matmul(psM[0:8, :], atb[:], wvTg[0:64, :],
                         start=True, stop=True,
                             skip_group_check=True, tile_position=(0, 0))
        nc.tensor.matmul(psM[64:72, :], atb[:], wvTg[0:64, :],
                         start=True, stop=True,
                             skip_group_check=True, tile_position=(0, 64))
        nc.scalar.copy(matrix2[0:8, :], psM[0:8, :])
        nc.scalar.copy(matrix2[64:72, :], psM[64:72, :])

        ksrow = sp1.tile([1, CQ], FP)
        nc.vector.tensor_reduce(
            ksrow[:], psks[:].rearrange("p (a b) -> p b a", b=CQ),
            axis=AX.X, op=ALU.add)
        kseps = sp1.tile([1, CQ], FP)
        nc.vector.tensor_scalar_add(kseps[:], ksrow[:], EPS)
        krow = sp1.tile([1, CQ], BF)
        nc.vector.tensor_copy(krow[:], kseps[:])
        psKO = sacc.tile([72, C], FP, tag="alg")
        nc.tensor.matmul(psKO[0:8, :], krow[:], ones_bf[0:1, :],
                         start=True, stop=True,
                             skip_group_check=True, tile_position=(0, 0))
        nc.tensor.matmul(psKO[64:72, :], krow[:], ones_bf[0:1, :],
                         start=True, stop=True,
                             skip_group_check=True, tile_position=(0, 64))
        nc.scalar.copy(kore[0:8, :], psKO[0:8, :])
        nc.scalar.copy(kore[64:72, :], psKO[64:72, :])

        ucol = sp1.tile([128, 1], FP)
        nc.vector.tensor_reduce(ucol[:], u_acc[:], axis=AX.X, op=ALU.add)
        ubf = sp1.tile([128, 1], BF)
        nc.vector.tensor_copy(ubf[:], ucol[:])
        psV = sacc.tile([C, 1], FP, tag="alg")
        nc.tensor.matmul(psV[:], wvTg[0:64, :], ubf[0:64, :],
                         start=True, stop=False,
                             skip_group_check=True, tile_position=(0, 0))
        nc.tensor.matmul(psV[:], wvTg[64:128, :], ubf[64:128, :],
                         start=False, stop=True,
                             skip_group_check=True, tile_position=(64, 0))
        vcol = sp1.tile([C, 1], FP)
        nc.vector.tensor_copy(vcol[:], psV[:])
        psVT = sacc.tile([1, C], FP, tag="alg")
        nc.tensor.transpose(psVT[0:1, :], vcol[:], iden[0:64, 0:64])
        nc.scalar.mul(vsrow[0:1, :], psVT[0:1, :], 1.0 / N)
        psVD = sacc.tile([65, C], FP, tag="alg2")
        nc.tensor.matmul(psVD[64:65, :], ones_bf[0:1, 0:1], vsrow[0:1, :],
                         start=True, stop=True,
                             skip_group_check=True, tile_position=(0, 64))
        nc.scalar.copy(vsrow[64:65, :], psVD[64:65, :])

    # =========================================================
    # Phase 3: Q path at quarter resolution (fused per chunk)
    # =========================================================
    with tc.tile_pool(name="qp", bufs=1) as qp, \
         tc.tile_pool(name="qin", bufs=2) as qin, \
         tc.tile_pool(name="qps2", bufs=2, space="PSUM") as qps2, \
         tc.tile_pool(name="qps1", bufs=1, space="PSUM") as qps1:

        h1p = qp.tile([128, PADN], BF)
        nc.gpsimd.memset(h1p[:], 0.0)

        h1p3 = h1p[:].rearrange("p (r w) -> p r w", w=PADW)  # [128, 66, 130]

        # ---- h1 = w_high conv into padded buffer (dup across halves) ----
        for qb in range(NQ // 2048):
            hc = qin.tile([128, 2048], BF, tag="hc")
            nc.gpsimd.dma_start(hc[:], high_d[:, qb * 2048:(qb + 1) * 2048])
            for qi in range(4):
                qc = qb * 4 + qi
                ps = qps2.tile([128, QCH], FP, tag="psH")
                nc.tensor.matmul(ps[:], whT[:],
                                 hc[:, qi * QCH:(qi + 1) * QCH],
                                 start=True, stop=True)
                half = qc // 16
                r0 = (qc * 4) % 64
                hrows = slice(half * 64, (half + 1) * 64)
                dst = h1p3[hrows, r0 + 1:r0 + 5, 1:1 + WQ]
                src = ps[hrows, :].rearrange("p (r w) -> p r w", w=WQ)
                nc.scalar.copy(dst, src)
                if qc == 15:  # A last qrow (63) -> B pad row 0
                    nc.scalar.copy(h1p3[64:128, 0:1, 1:1 + WQ],
                                   ps[64:128, 3 * WQ:4 * WQ].unsqueeze(1))
                if qc == 16:  # B first qrow (64) -> A pad row 65
                    nc.scalar.copy(h1p3[0:64, 65:66, 1:1 + WQ],
                                   ps[0:64, 0:WQ].unsqueeze(1))

        # ---- fused quarter-res chunk loop ----
        for qc in range(NQH // QCH):
            sl = slice(qc * QCH, (qc + 1) * QCH)
            r0 = qc * 4
            # conv3x3 (A and B halves concurrently on PE)
            psC = qps2.tile([128, QCH], FP, tag="psC")
            for tap in range(9):
                ky, kx = tap // 3, tap % 3
                for half in range(2):
                    hrows = slice(half * 64, (half + 1) * 64)
                    rhs = h1p3[hrows, r0 + ky:r0 + ky + 4, kx:kx + WQ]
                    nc.tensor.matmul(
                        psC[hrows, :].rearrange("p (r w) -> p r w", w=WQ),
                        wup[hrows, tap * C:(tap + 1) * C], rhs,
                        start=(tap == 0), stop=(tap == 8),
                        skip_group_check=True,
                        tile_position=(half * 64, half * 64))
            h2c = qin.tile([128, QCH], BF, tag="h2c")
            nc.scalar.activation(h2c[:], psC[:], AF.Relu,
                                 bias=bnt[:, 1:2], scale=bnt[:, 0:1])

            # q_pre [72, 512]
            psQ = qps1.tile([72, QCH], FP, tag="psQ")
            nc.tensor.matmul(psQ[0:64, :], wqT[0:64, :], h2c[0:64, :],
                             start=True, stop=True,
                             skip_group_check=True, tile_position=(0, 0))
            nc.tensor.matmul(psQ[64:72, :], wqT[64:128, 0:CQ], h2c[64:128, :],
                             start=True, stop=True,
                             skip_group_check=True, tile_position=(64, 64))
            qpre = qin.tile([72, QCH], BF, tag="qpre")
            nc.scalar.copy(qpre[:], psQ[:])

            # qsq broadcast + Nsq
            qsqc = qin.tile([72, QCH], BF, tag="qsqc")
            nc.scalar.square(qsqc[:], qpre[:])
            psB = qps1.tile([128, QCH], FP, tag="psB")
            nc.tensor.matmul(psB[0:64, :], ones_bf[0:8, :], qsqc[0:8, :],
                             start=True, stop=True,
                             skip_group_check=True, tile_position=(0, 0))
            nc.tensor.matmul(psB[64:128, :], ones_bf[64:72, :], qsqc[64:72, :],
                             start=True, stop=True,
                             skip_group_check=True, tile_position=(64, 64))
            nsq = qin.tile([128, QCH], BF, tag="nsq")
            nc.scalar.activation(nsq[:], psB[:], AF.Sqrt,
                                 scale=float(N) * float(N))

            # num = gamma*(ms_pre + sq*vs) ; den = N*sq + d_pre
            psN = qps1.tile([128, QCH], FP, tag="psN")
            nc.tensor.matmul(psN[0:64, :], matrix2[0:8, :], qpre[0:8, :],
                             start=True, stop=False,
                             skip_group_check=True, tile_position=(0, 0))
            nc.tensor.matmul(psN[64:128, :], matrix2[64:72, :], qpre[64:72, :],
                             start=True, stop=False,
                             skip_group_check=True, tile_position=(64, 64))
            nc.tensor.matmul(psN[0:64, :], vsrow[0:1, :], nsq[0:1, :],
                             start=False, stop=True,
                             skip_group_check=True, tile_position=(0, 0))
            nc.tensor.matmul(psN[64:128, :], vsrow[64:65, :], nsq[64:65, :],
                             start=False, stop=True,
                             skip_group_check=True, tile_position=(64, 64))
            psD = qps1.tile([128, QCH], FP, tag="psD")
            nc.tensor.matmul(psD[0:64, :], kore[0:8, :], qpre[0:8, :],
                             start=True, stop=True,
                             skip_group_check=True, tile_position=(0, 0))
            nc.tensor.matmul(psD[64:128, :], kore[64:72, :], qpre[64:72, :],
                             start=True, stop=True,
                             skip_group_check=True, tile_position=(64, 64))
            dn = qin.tile([128, QCH], FP, tag="dn")
            nc.vector.tensor_add(dn[:], psD[:], nsq[:])
            tl = qin.tile([128, QCH], FP, tag="tl")
            nc.vector.reciprocal(tl[:], dn[:])
            wvc = qin.tile([128, QCH], BF, tag="wvc")
            nc.vector.tensor_mul(wvc[:], psN[:], tl[:])

            # upsample 2x + residual add in place + store (4 qrows per chunk)
            for j in range(4):
                y = qc * 4 + j
                dst = lowf[:, y * 2 * W:(y + 1) * 2 * W].rearrange(
                    "p (r x d) -> p r x d", r=2, d=2)
                src = wvc[:, j * WQ:(j + 1) * WQ] \
                    .unsqueeze(1).broadcast_to([128, 2, WQ]) \
                    .unsqueeze(3).broadcast_to([128, 2, WQ, 2])
                nc.vector.tensor_add(dst, dst, src)
            lo2 = qc * 4 * 2 * W
            nc.sync.dma_start(out_hc[:, :, lo2:lo2 + 2048],
                              lowf[:, lo2:lo2 + 2048])


def prep_consts(w_high, w_up, bn_gamma, bn_beta, bn_mean, bn_var,
                w_q, w_k, w_v, gamma):
    g = float(np.asarray(gamma).reshape(-1)[0])
    whT = np.ascontiguousarray(w_high.T)                      # [128, 64]
    whT_d = np.concatenate([whT, whT], axis=1).astype(ml_dtypes.bfloat16)
    taps = [np.ascontiguousarray(w_up[:, :, ky, kx].T)
            for ky in range(3) for kx in range(3)]
    wup_h = np.concatenate(taps, axis=1)                      # [64, 576]
    wup_d = np.concatenate([wup_h, wup_h], axis=0).astype(ml_dtypes.bfloat16)
    scale = bn_gamma / np.sqrt(bn_var + BN_EPS)
    shift = bn_beta - bn_mean * scale
    bn_h = np.stack([scale, shift], axis=1).astype(np.float32)
    bn_d = np.concatenate([bn_h, bn_h], axis=0)               # [128, 2]
    wqT = np.zeros((64, 64), np.float32)
    wqT[:, 0:8] = w_q.T
    wqT_d = np.concatenate([wqT, wqT], axis=0).astype(ml_dtypes.bfloat16)
    wkT = np.zeros((64, 32), np.float32)
    wkT[:, 0:8] = w_k.T
    wkT_d = np.concatenate([wkT, wkT], axis=0).astype(ml_dtypes.bfloat16)
    wvTg = np.ascontiguousarray((g * w_v).T)
    wvTg_d = np.concatenate([wvTg, wvTg], axis=0).astype(ml_dtypes.bfloat16)
    iden_d = np.eye(128, dtype=np.float32)
    return whT_d, wup_d, bn_d, wqT_d, wkT_d, wvTg_d, iden_d


_CACHE = {}


def make_ins_outs(nc):
    ins = [
        nc.dram_tensor("low", [C, N], FP, kind="ExternalInput").ap(),
        nc.dram_tensor("high", [CH, NQ], FP, kind="ExternalInput").ap(),
        nc.dram_tensor("whT", [128, 128], BF, kind="ExternalInput").ap(),
        nc.dram_tensor("wup", [128, 9 * C], BF, kind="ExternalInput").ap(),
        nc.dram_tensor("bn", [128, 2], FP, kind="ExternalInput").ap(),
        nc.dram_tensor("wqT", [128, C], BF, kind="ExternalInput").ap(),
        nc.dram_tensor("wkT", [128, 32], BF, kind="ExternalInput").ap(),
        nc.dram_tensor("wvTg", [128, C], BF, kind="ExternalInput").ap(),
        nc.dram_tensor("iden", [128, 128], FP, kind="ExternalInput").ap(),
    ]
    outs = [nc.dram_tensor("out", [C, N], FP, kind="ExternalOutput").ap()]
    return ins, outs


def _build_nc():
    if "nc" in _CACHE:
        return _CACHE["nc"]
    nc = bacc.Bacc("TRN2", target_bir_lowering=False, debug=False)
    ins, outs = make_ins_outs(nc)
    with tile.TileContext(nc) as tc:
        with ExitStack() as ctx:
            build_program(ctx, tc, outs, ins)
    nc.compile()
    _CACHE["nc"] = nc
    return nc


def make_in_maps(inputs):
    low = np.asarray(inputs["low"], np.float32)
    high = np.asarray(inputs["high"], np.float32)
    consts = prep_consts(
        np.asarray(inputs["w_high"], np.float32),
        np.asarray(inputs["w_up"], np.float32),
        np.asarray(inputs["bn_gamma"], np.float32),
        np.asarray(inputs["bn_beta"], np.float32),
        np.asarray(inputs["bn_mean"], np.float32),
        np.asarray(inputs["bn_var"], np.float32),
        np.asarray(inputs["w_q"], np.float32),
        np.asarray(inputs["w_k"], np.float32),
        np.asarray(inputs["w_v"], np.float32),
        np.asarray(inputs["gamma"], np.float32),
    )
    names = ["whT", "wup", "bn", "wqT", "wkT", "wvTg", "iden"]
    B = low.shape[0]
    in_maps = []
    for b in range(B):
        m = {"low": np.ascontiguousarray(low[b].reshape(C, N)),
             "high": np.ascontiguousarray(high[b].reshape(CH, NQ))}
        m.update({k: v for k, v in zip(names, consts)})
        in_maps.append(m)
    return in_maps


def _get_runner(n_cores=8):
    """Build (once) a cached sharded PJRT executable for the Bass program."""
    if "runner" in _CACHE:
        return _CACHE["runner"]
    import jax
    import jax.numpy as jnp
    from jax.sharding import Mesh, PartitionSpec
    from jax.experimental.shard_map import shard_map
    from concourse import bass2jax, mybir as _mb
    bass2jax.install_neuronx_cc_hook()

    nc = _build_nc()
    pname = nc.partition_id_tensor.name if nc.partition_id_tensor else None
    in_names, out_names, out_avals, zero_shapes = [], [], [], []
    for alloc in nc.m.functions[0].allocations:
        if not isinstance(alloc, _mb.MemoryLocationSet):
            continue
        name = alloc.memorylocations[0].name
        if alloc.kind == "ExternalInput":
            if name != pname:
                in_names.append(name)
        elif alloc.kind == "ExternalOutput":
            out_names.append(name)
            shape = tuple(alloc.tensor_shape)
            dtype = _mb.dt.np(alloc.dtype)
            out_avals.append(jax.core.ShapedArray(shape, dtype))
            zero_shapes.append((shape, dtype))
    n_params = len(in_names)
    all_names = in_names + out_names
    if pname is not None:
        all_names = all_names + [pname]
    donate = tuple(range(n_params, n_params + len(out_names)))

    def _body(*args):
        operands = list(args)
        if pname is not None:
            operands.append(bass2jax.partition_id_tensor())
        outs = bass2jax._bass_exec_p.bind(
            *operands,
            out_avals=tuple(out_avals),
            in_names=tuple(all_names),
            out_names=tuple(out_names),
            lowering_input_output_aliases=(),
            sim_require_finite=True,
            sim_require_nnan=True,
            nc=nc,
        )
        return tuple(outs)

    devices = jax.devices()[:n_cores]
    mesh = Mesh(np.asarray(devices), ("core",))
    n_args = n_params + len(out_names)
    sharded = jax.jit(
        shard_map(_body, mesh=mesh,
                  in_specs=(PartitionSpec("core"),) * n_args,
                  out_specs=(PartitionSpec("core"),) * len(out_names),
                  check_rep=False),
        donate_argnums=donate, keep_unused=True)
    runner = {"fn": sharded, "in_names": in_names, "out_names": out_names,
              "zero_shapes": zero_shapes, "n_cores": n_cores, "mesh": mesh}
    _CACHE["runner"] = runner
    return runner


def run_concat(concat_in, runner):
    """Run the sharded executable on pre-concatenated inputs."""
    n_cores = runner["n_cores"]
    zeros = [np.zeros((n_cores * sh[0], *sh[1:]), dt)
             for sh, dt in runner["zero_shapes"]]
    return runner["fn"](*concat_in, *zeros)


def make_concat_inputs(inputs, runner):
    in_maps = make_in_maps(inputs)
    return [np.concatenate([m[name] for m in in_maps], axis=0)
            for name in runner["in_names"]]


def kernel(**inputs):
    runner = _get_runner()
    concat_in = make_concat_inputs(inputs, runner)
    out_arrs = run_concat(concat_in, runner)
    n = runner["n_cores"]
    out = np.asarray(out_arrs[0]).reshape(n, C, W, W)
    return out.astype(np.float32)



# revision 2
# speedup vs baseline: 3.6620x; 3.6620x over previous
"""Trainium2 Bass kernel for nn_BA_Model_46471546142978 (linear-attention fusion).

Self-contained: takes FULL inputs, shards batch across 8 NeuronCores (one
batch element per core), runs one SPMD Bass program, gathers FULL output.

Algorithm per core (batch b), restructured to avoid materializing K/V:
  A      = sum_n r_n * K[:,n] * low[:,n]^T          (8x64,  = Wk @ S)
  k_sum  = sum_n r_n * K[:,n] + eps                 (8,)
  matrix = A @ (gamma*Wv)^T                         (8x64)
  vs     = gamma*Wv @ u,  u = sum_n low[:,n]        (64,)
with r_n = 1/||Wk low_n||. The Q path runs at quarter resolution (128x128)
since nearest-upsample commutes with the per-pixel 1x1 convs + normalize:
  wv = gamma*(ms_pre + sq*vs) / (N*sq + d_pre),  sq = ||q_pre||
  out = low + upsample2x(wv)

Layout: "stacked halves" - partitions 0:63 = 64 channels of spatial top half
(rows Y<128), partitions 64:127 = bottom half. Full-res DVE/ACT ops run on
[128, *] tiles; matmuls run as concurrent tile_position pairs.
"""
from contextlib import ExitStack

import numpy as np
import ml_dtypes

import concourse.bass as bass
import concourse.bacc as bacc
import concourse.tile as tile
from concourse import mybir

FP = mybir.dt.float32
BF = mybir.dt.bfloat16
FR = mybir.dt.float32r
AF = mybir.ActivationFunctionType
ALU = mybir.AluOpType
AX = mybir.AxisListType

C = 64          # channels
CH = 128        # high channels
N = 65536       # full-res pixels (256x256)
NH = 32768      # per half
NQ = 16384      # quarter-res pixels (128x128)
NQH = 8192      # per half
W = 256         # full-res row width
WQ = 128        # quarter-res row width
CQ = 8          # q/k channels
EPS = 1e-6
BN_EPS = 1e-5

SEC = 8                 # sections for the low pass
SECN = NH // SEC        # 4096 half-pixels per section
SBLK = SECN // 128      # 32 transpose blocks per section
KP = 48                 # padded K partition count for DMA transpose (mult 16)
QCH = 512               # chunk size in quarter-res loops
PADW = WQ + 2           # 130, padded conv row
PADR = WQ // 2 + 2      # 66 padded rows per half
PADN = PADW * PADR      # per-half padded h1 buffer free size


def build_program(ctx: ExitStack, tc: tile.TileContext, outs, ins):
    nc = tc.nc
    (out_d,) = outs
    (low_d, high_d, whT_d, wup_d, bn_d, wqT_d, wkT_d, wvTg_d, iden_d) = ins

    consts = ctx.enter_context(tc.tile_pool(name="consts", bufs=1))
    resid = ctx.enter_context(tc.tile_pool(name="resid", bufs=1))

    # ---------------- constants ----------------
    whT = consts.tile([128, 128], BF)
    nc.sync.dma_start(whT[:], whT_d[:])
    wup = consts.tile([128, 9 * C], BF)
    nc.sync.dma_start(wup[:], wup_d[:])
    bnt = consts.tile([128, 2], FP)
    nc.sync.dma_start(bnt[:], bn_d[:])
    wqT = consts.tile([128, C], BF)
    nc.sync.dma_start(wqT[:], wqT_d[:])
    wkT = consts.tile([128, 32], BF)
    nc.sync.dma_start(wkT[:], wkT_d[:])
    wvTg = consts.tile([128, C], BF)
    nc.sync.dma_start(wvTg[:], wvTg_d[:])
    iden = consts.tile([128, 128], FP)
    nc.sync.dma_start(iden[:], iden_d[:])
    ones_bf = consts.tile([128, C], BF)
    nc.gpsimd.memset(ones_bf[:], 1.0)

    u_acc = consts.tile([128, SEC], FP)
    matrix2 = consts.tile([72, C], BF)    # rows 0:8 / 64:72 = gamma*matrix
    kore = consts.tile([72, C], BF)       # rows 0:8 / 64:72 = k_sum bcast cols
    vsrow = consts.tile([65, C], BF)      # rows 0 / 64 = gamma*vs/N

    lowf = resid.tile([128, NH], FP)      # resident low; becomes the output
    # stacked-halves DRAM views: [half, channel, n]
    low_hc = low_d.rearrange("c (h n) -> h c n", h=2)
    out_hc = out_d.rearrange("c (h n) -> h c n", h=2)

    # =========================================================
    # Phase 1: low pass - accumulate A [8,64], k_sum [8,1], u
    # =========================================================
    with tc.tile_pool(name="sp", bufs=2) as sp, \
         tc.tile_pool(name="sp1", bufs=1) as sp1, \
         tc.tile_pool(name="spp", bufs=2, space="PSUM") as spp, \
         tc.tile_pool(name="sacc", bufs=1, space="PSUM") as sacc:

        psA = sacc.tile([CQ, C], FP)
        psks = sacc.tile([1, 2 * SBLK * CQ], FP)
        ones_col = sp1.tile([128, 1], BF)
        nc.gpsimd.memset(ones_col[:], 1.0)

        n_mm = SEC * SBLK * 2
        mm_i = 0
        for s in range(SEC):
            lo = s * SECN
            hi = lo + SECN
            nc.sync.dma_start(lowf[:, lo:hi], low_hc[:, :, lo:hi])

            lowbf = sp.tile([128, SECN], BF, tag="lowbf")
            nc.scalar.activation(lowbf[:], lowf[:, lo:hi], AF.Copy,
                                 accum_out=u_acc[:, s:s + 1])

            kbf = sp.tile([KP, SECN], BF, tag="kbf")
            for cc in range(SECN // 1024):
                ps = spp.tile([KP, 1024], FP, tag="psK")
                for q2 in range(2):
                    sl5 = slice(q2 * QCH, (q2 + 1) * QCH)
                    sl = slice(cc * 1024 + q2 * QCH, cc * 1024 + (q2 + 1) * QCH)
                    nc.tensor.matmul(ps[0:32, sl5], wkT[0:64, 0:32],
                                     lowbf[0:64, sl], start=True, stop=True,
                                     skip_group_check=True,
                                     tile_position=(0, 0))
                    nc.tensor.matmul(ps[32:48, sl5], wkT[64:128, 0:16],
                                     lowbf[64:128, sl], start=True, stop=True,
                                     skip_group_check=True,
                                     tile_position=(64, 32))
                nc.vector.tensor_copy(kbf[0:KP, cc * 1024:(cc + 1) * 1024], ps[:])

            lowT = sp.tile([128, SBLK, 128], BF, tag="lowT")
            nc.sync.dma_start(lowT[:], lowbf[:], transpose=True)
            kT = sp.tile([128, SBLK, KP], BF, tag="kT")
            nc.sync.dma_start(kT[:], kbf[:], transpose=True)

            sqk = sp.tile([128, 2 * SBLK, CQ], BF, tag="sqk")
            nc.scalar.square(sqk[:, 0:SBLK, :], kT[:, :, 0:8])
            nc.scalar.square(sqk[:, SBLK:2 * SBLK, :], kT[:, :, 32:40])
            ksq = sp.tile([128, 2 * SBLK], FP, tag="ksq")
            nc.vector.tensor_reduce(ksq[:], sqk[:], axis=AX.X, op=ALU.add)
            sqr = sp.tile([128, 2 * SBLK], FP, tag="sqr")
            nc.scalar.sqrt(sqr[:], ksq[:])
            rr = sp.tile([128, 2 * SBLK], FP, tag="rr")
            nc.vector.reciprocal(rr[:], sqr[:])
            rrb = sp.tile([128, 2 * SBLK], BF, tag="rrb")
            nc.vector.tensor_copy(rrb[:], rr[:])

            ktr = sp.tile([128, 2 * SBLK, CQ], BF, tag="ktr")
            nc.vector.tensor_mul(
                ktr[:, 0:SBLK, :], kT[:, :, 0:8],
                rrb[:, 0:SBLK].unsqueeze(2).broadcast_to([128, SBLK, CQ]))
            nc.vector.tensor_mul(
                ktr[:, SBLK:2 * SBLK, :], kT[:, :, 32:40],
                rrb[:, SBLK:2 * SBLK].unsqueeze(2).broadcast_to([128, SBLK, CQ]))

            nc.tensor.matmul(psks[:], ones_col[:],
                             ktr[:].rearrange("p a b -> p (a b)"),
                             start=(s == 0), stop=(s == SEC - 1),
                             skip_group_check=True)
            for blk in range(SBLK):
                for half in range(2):
                    lhs = ktr[:, half * SBLK + blk, :]
                    rhs = lowT[:, blk, half * 64:(half + 1) * 64]
                    st = mm_i == 0
                    fin = mm_i == n_mm - 1
                    nc.tensor.matmul(psA[:], lhs, rhs, start=st, stop=fin,
                                     skip_group_check=True)
                    mm_i += 1

        # ----- small algebra -----
        asb = sp1.tile([CQ, C], FP)
        nc.vector.tensor_copy(asb[:], psA[:])
        psAT = sacc.tile([C, CQ], FP, tag="alg")
        nc.tensor.transpose(psAT[:], asb[:], iden[0:8, 0:8])
        atb = sp1.tile([C, CQ], BF)
        nc.scalar.copy(atb[:], psAT[:])

        psM = sacc.tile([72, C], FP, tag="alg")
        nc.tensor.matmul(psM[0:8, :], atb[:], wvTg[0:64, :],
                         start=True, stop=True,
                             skip_group_check=True, tile_position=(0, 0))
        nc.tensor.matmul(psM[64:72, :], atb[:], wvTg[0:64, :],
                         start=True, stop=True,
                             skip_group_check=True, tile_position=(0, 64))
        nc.scalar.copy(matrix2[0:8, :], psM[0:8, :])
        nc.scalar.copy(matrix2[64:72, :], psM[64:72, :])

        ksrow = sp1.tile([1, CQ], FP)
        nc.vector.tensor_reduce(
            ksrow[:], psks[:].rearrange("p (a b) -> p b a", b=CQ),
            axis=AX.X, op=ALU.add)
        kseps = sp1.tile([1, CQ], FP)
        nc.vector.tensor_scalar_add(kseps[:], ksrow[:], EPS)
        krow = sp1.tile([1, CQ], BF)
        nc.vector.tensor_copy(krow[:], kseps[:])
        psKO = sacc.tile([72, C], FP, tag="alg")
        nc.tensor.matmul(psKO[0:8, :], krow[:], ones_bf[0:1, :],
                         start=True, stop=True,
                             skip_group_check=True, tile_position=(0, 0))
        nc.tensor.matmul(psKO[64:72, :], krow[:], ones_bf[0:1, :],
                         start=True, stop=True,
                             skip_group_check=True, tile_position=(0, 64))
        nc.scalar.copy(kore[0:8, :], psKO[0:8, :])
        nc.scalar.copy(kore[64:72, :], psKO[64:72, :])

        ucol = sp1.tile([128, 1], FP)
        nc.vector.tensor_reduce(ucol[:], u_acc[:], axis=AX.X, op=ALU.add)
        ubf = sp1.tile([128, 1], BF)
        nc.vector.tensor_copy(ubf[:], ucol[:])
        psV = sacc.tile([C, 1], FP, tag="alg")
        nc.tensor.matmul(psV[:], wvTg[0:64, :], ubf[0:64, :],
                         start=True, stop=False,
                             skip_group_check=True, tile_position=(0, 0))
        nc.tensor.matmul(psV[:], wvTg[64:128, :], ubf[64:128, :],
                         start=False, stop=True,
                             skip_group_check=True, tile_position=(64, 0))
        vcol = sp1.tile([C, 1], FP)
        nc.vector.tensor_copy(vcol[:], psV[:])
        psVT = sacc.tile([1, C], FP, tag="alg")
        nc.tensor.transpose(psVT[0:1, :], vcol[:], iden[0:64, 0:64])
        nc.scalar.mul(vsrow[0:1, :], psVT[0:1, :], 1.0 / N)
        psVD = sacc.tile([65, C], FP, tag="alg2")
        nc.tensor.matmul(psVD[64:65, :], ones_bf[0:1, 0:1], vsrow[0:1, :],
                         start=True, stop=True,
                             skip_group_check=True, tile_position=(0, 64))
        nc.scalar.copy(vsrow[64:65, :], psVD[64:65, :])

    # =========================================================
    # Phase 3: Q path at quarter resolution (fused per chunk)
    # =========================================================
    with tc.tile_pool(name="qp", bufs=1) as qp, \
         tc.tile_pool(name="qin", bufs=2) as qin, \
         tc.tile_pool(name="qps2", bufs=2, space="PSUM") as qps2, \
         tc.tile_pool(name="qps1", bufs=1, space="PSUM") as qps1:

        h1p = qp.tile([128, PADN], BF)
        nc.gpsimd.memset(h1p[:], 0.0)

        h1p3 = h1p[:].rearrange("p (r w) -> p r w", w=PADW)  # [128, 66, 130]

        # ---- h1 = w_high conv into padded buffer (dup across halves) ----
        for qb in range(NQ // 2048):
            hc = qin.tile([128, 2048], BF, tag="hc")
            nc.gpsimd.dma_start(hc[:], high_d[:, qb * 2048:(qb + 1) * 2048])
            for qi in range(4):
                qc = qb * 4 + qi
                ps = qps2.tile([128, QCH], FP, tag="psH")
                nc.tensor.matmul(ps[:], whT[:],
                                 hc[:, qi * QCH:(qi + 1) * QCH],
                                 start=True, stop=True)
                half = qc // 16
                r0 = (qc * 4) % 64
                hrows = slice(half * 64, (half + 1) * 64)
                dst = h1p3[hrows, r0 + 1:r0 + 5, 1:1 + WQ]
                src = ps[hrows, :].rearrange("p (r w) -> p r w", w=WQ)
                nc.scalar.copy(dst, src)
                if qc == 15:  # A last qrow (63) -> B pad row 0
                    nc.scalar.copy(h1p3[64:128, 0:1, 1:1 + WQ],
                                   ps[64:128, 3 * WQ:4 * WQ].unsqueeze(1))
                if qc == 16:  # B first qrow (64) -> A pad row 65
                    nc.scalar.copy(h1p3[0:64, 65:66, 1:1 + WQ],
                                   ps[0:64, 0:WQ].unsqueeze(1))

        # ---- fused quarter-res chunk loop ----
        for qc in range(NQH // QCH):
            sl = slice(qc * QCH, (qc + 1) * QCH)
            r0 = qc * 4
            # conv3x3 (A and B halves concurrently on PE)
            psC = qps2.tile([128, QCH], FP, tag="psC")
            for tap in range(9):
                ky, kx = tap // 3, tap % 3
                for half in range(2):
                    hrows = slice(half * 64, (half + 1) * 64)
                    rhs = h1p3[hrows, r0 + ky:r0 + ky + 4, kx:kx + WQ]
                    nc.tensor.matmul(
                        psC[hrows, :].rearrange("p (r w) -> p r w", w=WQ),
                        wup[hrows, tap * C:(tap + 1) * C], rhs,
                        start=(tap == 0), stop=(tap == 8),
                        skip_group_check=True,
                        tile_position=(half * 64, half * 64))
            h2c = qin.tile([128, QCH], BF, tag="h2c")
            nc.scalar.activation(h2c[:], psC[:], AF.Relu,
                                 bias=bnt[:, 1:2], scale=bnt[:, 0:1])

            # q_pre [72, 512]
            psQ = qps1.tile([72, QCH], FP, tag="psQ")
            nc.tensor.matmul(psQ[0:64, :], wqT[0:64, :], h2c[0:64, :],
                             start=True, stop=True,
                             skip_group_check=True, tile_position=(0, 0))
            nc.tensor.matmul(psQ[64:72, :], wqT[64:128, 0:CQ], h2c[64:128, :],
                             start=True, stop=True,
                             skip_group_check=True, tile_position=(64, 64))
            qpre = qin.tile([72, QCH], BF, tag="qpre")
            nc.scalar.copy(qpre[:], psQ[:])

            # qsq broadcast + Nsq
            qsqc = qin.tile([72, QCH], BF, tag="qsqc")
            nc.scalar.square(qsqc[:], qpre[:])
            psB = qps1.tile([128, QCH], FP, tag="psB")
            nc.tensor.matmul(psB[0:64, :], ones_bf[0:8, :], qsqc[0:8, :],
                             start=True, stop=True,
                             skip_group_check=True, tile_position=(0, 0))
            nc.tensor.matmul(psB[64:128, :], ones_bf[64:72, :], qsqc[64:72, :],
                             start=True, stop=True,
                             skip_group_check=True, tile_position=(64, 64))
            nsq = qin.tile([128, QCH], BF, tag="nsq")
            nc.scalar.activation(nsq[:], psB[:], AF.Sqrt,
                                 scale=float(N) * float(N))

            # num = gamma*(ms_pre + sq*vs) ; den = N*sq + d_pre
            psN = qps1.tile([128, QCH], FP, tag="psN")
            nc.tensor.matmul(psN[0:64, :], matrix2[0:8, :], qpre[0:8, :],
                             start=True, stop=False,
                             skip_group_check=True, tile_position=(0, 0))
            nc.tensor.matmul(psN[64:128, :], matrix2[64:72, :], qpre[64:72, :],
                             start=True, stop=False,
                             skip_group_check=True, tile_position=(64, 64))
            nc.tensor.matmul(psN[0:64, :], vsrow[0:1, :], nsq[0:1, :],
                             start=False, stop=True,
                             skip_group_check=True, tile_position=(0, 0))
            nc.tensor.matmul(psN[64:128, :], vsrow[64:65, :], nsq[64:65, :],
                             start=False, stop=True,
                             skip_group_check=True, tile_position=(64, 64))
            psD = qps1.tile([128, QCH], FP, tag="psD")
            nc.tensor.matmul(psD[0:64, :], kore[0:8, :], qpre[0:8, :],
                             start=True, stop=True,
                             skip_group_check=True, tile_position=(0, 0))
            nc.tensor.matmul(psD[64:128, :], kore[64:72, :], qpre[64:72, :],
                             start=True, stop=True,
                             skip_group_check=True, tile_position=(64, 64))
            dn = qin.tile([128, QCH], FP, tag="dn")
            nc.vector.tensor_add(dn[:], psD[:], nsq[:])
            tl = qin.tile([128, QCH], FP, tag="tl")
            nc.vector.reciprocal(tl[:], dn[:])
            wvc = qin.tile([128, QCH], BF, tag="wvc")
            nc.vector.tensor_mul(wvc[:], psN[:], tl[:])

            # upsample 2x + residual add in place + store (4 qrows per chunk)
            for j in range(4):
                y = qc * 4 + j
                dst = lowf[:, y * 2 * W:(y + 1) * 2 * W].rearrange(
                    "p (r x d) -> p r x d", r=2, d=2)
                src = wvc[:, j * WQ:(j + 1) * WQ] \
                    .unsqueeze(1).broadcast_to([128, 2, WQ]) \
                    .unsqueeze(3).broadcast_to([128, 2, WQ, 2])
                nc.vector.tensor_add(dst, dst, src)
            lo2 = qc * 4 * 2 * W
            nc.sync.dma_start(out_hc[:, :, lo2:lo2 + 2048],
                              lowf[:, lo2:lo2 + 2048])


def prep_consts(w_high, w_up, bn_gamma, bn_beta, bn_mean, bn_var,
                w_q, w_k, w_v, gamma):
    g = float(np.asarray(gamma).reshape(-1)[0])
    whT = np.ascontiguousarray(w_high.T)                      # [128, 64]
    whT_d = np.concatenate([whT, whT], axis=1).astype(ml_dtypes.bfloat16)
    taps = [np.ascontiguousarray(w_up[:, :, ky, kx].T)
            for ky in range(3) for kx in range(3)]
    wup_h = np.concatenate(taps, axis=1)                      # [64, 576]
    wup_d = np.concatenate([wup_h, wup_h], axis=0).astype(ml_dtypes.bfloat16)
    scale = bn_gamma / np.sqrt(bn_var + BN_EPS)
    shift = bn_beta - bn_mean * scale
    bn_h = np.stack([scale, shift], axis=1).astype(np.float32)
    bn_d = np.concatenate([bn_h, bn_h], axis=0)               # [128, 2]
    wqT = np.zeros((64, 64), np.float32)
    wqT[:, 0:8] = w_q.T
    wqT_d = np.concatenate([wqT, wqT], axis=0).astype(ml_dtypes.bfloat16)
    wkT = np.zeros((64, 32), np.float32)
    wkT[:, 0:8] = w_k.T
    wkT_d = np.concatenate([wkT, wkT], axis=0).astype(ml_dtypes.bfloat16)
    wvTg = np.ascontiguousarray((g * w_v).T)
    wvTg_d = np.concatenate([wvTg, wvTg], axis=0).astype(ml_dtypes.bfloat16)
    iden_d = np.eye(128, dtype=np.float32)
    return whT_d, wup_d, bn_d, wqT_d, wkT_d, wvTg_d, iden_d


_CACHE = {}


def make_ins_outs(nc):
    ins = [
        nc.dram_tensor("low", [C, N], FP, kind="ExternalInput").ap(),
        nc.dram_tensor("high", [CH, NQ], FP, kind="ExternalInput").ap(),
        nc.dram_tensor("whT", [128, 128], BF, kind="ExternalInput").ap(),
        nc.dram_tensor("wup", [128, 9 * C], BF, kind="ExternalInput").ap(),
        nc.dram_tensor("bn", [128, 2], FP, kind="ExternalInput").ap(),
        nc.dram_tensor("wqT", [128, C], BF, kind="ExternalInput").ap(),
        nc.dram_tensor("wkT", [128, 32], BF, kind="ExternalInput").ap(),
        nc.dram_tensor("wvTg", [128, C], BF, kind="ExternalInput").ap(),
        nc.dram_tensor("iden", [128, 128], FP, kind="ExternalInput").ap(),
    ]
    outs = [nc.dram_tensor("out", [C, N], FP, kind="ExternalOutput").ap()]
    return ins, outs


def _build_nc():
    if "nc" in _CACHE:
        return _CACHE["nc"]
    nc = bacc.Bacc("TRN2", target_bir_lowering=False, debug=False)
    ins, outs = make_ins_outs(nc)
    with tile.TileContext(nc) as tc:
        with ExitStack() as ctx:
            build_program(ctx, tc, outs, ins)
    nc.compile()
    _CACHE["nc"] = nc
    return nc


def make_in_maps(inputs):
    low = np.asarray(inputs["low"], np.float32)
    high = np.asarray(inputs["high"], np.float32)
    consts = prep_consts(
        np.asarray(inputs["w_high"], np.float32),
        np.asarray(inputs["w_up"], np.float32),
        np.asarray(inputs["bn_gamma"], np.float32),
        np.asarray(inputs["bn_beta"], np.float32),
        np.asarray(inputs["bn_mean"], np.float32),
        np.asarray(inputs["bn_var"], np.float32),
        np.asarray(inputs["w_q"], np.float32),
        np.asarray(inputs["w_k"], np.float32),
        np.asarray(inputs["w_v"], np.float32),
        np.asarray(inputs["gamma"], np.float32),
    )
    names = ["whT", "wup", "bn", "wqT", "wkT", "wvTg", "iden"]
    B = low.shape[0]
    in_maps = []
    for b in range(B):
        m = {"low": np.ascontiguousarray(low[b].reshape(C, N)),
             "high": np.ascontiguousarray(high[b].reshape(CH, NQ))}
        m.update({k: v for k, v in zip(names, consts)})
        in_maps.append(m)
    return in_maps


def _get_runner(n_cores=8):
    """Build (once) a cached sharded PJRT executable for the Bass program.

    Outputs are fully written by the kernel, so no donated zero buffers
    are passed; the jit is AOT-compiled with bass fast dispatch.
    """
    if "runner" in _CACHE:
        return _CACHE["runner"]
    import jax
    from jax.sharding import Mesh, PartitionSpec, NamedSharding
    from jax.experimental.shard_map import shard_map
    from concourse import bass2jax, mybir as _mb
    bass2jax.install_neuronx_cc_hook()

    nc = _build_nc()
    pname = nc.partition_id_tensor.name if nc.partition_id_tensor else None
    in_names, in_shapes, out_names, out_avals = [], [], [], []
    for alloc in nc.m.functions[0].allocations:
        if not isinstance(alloc, _mb.MemoryLocationSet):
            continue
        name = alloc.memorylocations[0].name
        if alloc.kind == "ExternalInput":
            if name != pname:
                in_names.append(name)
                in_shapes.append((tuple(alloc.tensor_shape),
                                  _mb.dt.np(alloc.dtype)))
        elif alloc.kind == "ExternalOutput":
            out_names.append(name)
            shape = tuple(alloc.tensor_shape)
            dtype = _mb.dt.np(alloc.dtype)
            out_avals.append(jax.core.ShapedArray(shape, dtype))
    n_params = len(in_names)
    all_names = list(in_names)
    if pname is not None:
        all_names = all_names + [pname]

    def _body(*args):
        operands = list(args)
        if pname is not None:
            operands.append(bass2jax.partition_id_tensor())
        outs = bass2jax._bass_exec_p.bind(
            *operands,
            out_avals=tuple(out_avals),
            in_names=tuple(all_names),
            out_names=tuple(out_names),
            lowering_input_output_aliases=(),
            sim_require_finite=True,
            sim_require_nnan=True,
            nc=nc,
        )
        return tuple(outs)

    devices = jax.devices()[:n_cores]
    mesh = Mesh(np.asarray(devices), ("core",))
    sm = shard_map(_body, mesh=mesh,
                   in_specs=(PartitionSpec("core"),) * n_params,
                   out_specs=(PartitionSpec("core",),) * len(out_names),
                   check_rep=False)
    sh = NamedSharding(mesh, PartitionSpec("core"))
    arg_structs = [jax.ShapeDtypeStruct((n_cores * s[0], *s[1:]), d,
                                        sharding=sh) for s, d in in_shapes]
    try:
        fn = bass2jax.fast_dispatch_compile(
            lambda: jax.jit(sm, keep_unused=True).lower(
                *arg_structs).compile())
    except Exception:
        fn = jax.jit(sm, keep_unused=True)
    runner = {"fn": fn, "in_names": in_names, "out_names": out_names,
              "zero_shapes": [], "n_cores": n_cores, "mesh": mesh}
    _CACHE["runner"] = runner
    return runner


def run_concat(concat_in, runner):
    """Run the sharded executable on pre-concatenated inputs."""
    return runner["fn"](*concat_in)


def make_concat_inputs(inputs, runner):
    in_maps = make_in_maps(inputs)
    return [np.concatenate([m[name] for m in in_maps], axis=0)
            for name in runner["in_names"]]


def kernel(**inputs):
    runner = _get_runner()
    concat_in = make_concat_inputs(inputs, runner)
    out_arrs = run_concat(concat_in, runner)
    n = runner["n_cores"]
    out = np.asarray(out_arrs[0]).reshape(n, C, W, W)
    return out.astype(np.float32)
